# revision 1
# baseline (speedup 1.0000x reference)
"""MoE LoRA delta kernel for Trainium2 (8 NeuronCores, data-parallel over tokens).

Computation (per token t):
    logits = x @ router_w.T                      [T, 4]
    gates  = top2-softmax(logits)                [T, 4]  (exactly 2 nonzero)
    mid    = x @ A_all.T                         [T, 64]   A_all[(e,r), d]
    delta  = (mid * expand(gates) * 4.0) @ B_all [T, D]    B_all[(e,r), d]

Kernel strategy per core (T_c = 1024 tokens):
  - W = concat([A_all, router_w]) -> [68, D]; host passes W.T [D, 68] so the
    stationary operand loads directly.  mm1 computes [68, T] = W @ x.T with
    fp32 accumulation; rows 64:68 are the router logits (exact-enough fp32 so
    the top-2 expert selection matches the fp32 reference).
  - x.T tiles produced on-chip with PE transpose-mode matmuls (fp32, exact).
  - Gating runs with tokens on partitions (small PE transposes of the logits),
    all in fp32: g_e = 1{t_e >= m2} * sigmoid(2*t_e - m2), t = l - max(l).
  - Gates expanded to (e,r) rows and scaled by 4.0 with a tiny selection
    matmul, then mid is scaled elementwise and fed to mm2 against B_all.
"""

import os
import sys

for _p in ("/opt/trn_rl_repo", "/root/.axon_site/_ro/trn_rl_repo"):
    if os.path.isdir(_p) and _p not in sys.path:
        sys.path.insert(0, _p)

import numpy as np
from contextlib import ExitStack

import concourse.bass as bass
import concourse.bacc as bacc
import concourse.mybir as mybir
import concourse.tile as tile

N_CORES = 8
B_, S, D = 4, 2048, 3840
T_FULL = B_ * S                 # 8192
T_C = T_FULL // N_CORES         # 1024 tokens per core
E, R = 4, 16
ER = E * R                      # 64
M_W = ER + E                    # 68 = A rows + router rows
LORA_SCALE = 16.0 / np.sqrt(16.0)   # 4.0

GROUP = 256                     # tokens per mm1 group
TPG = GROUP // 128              # token tiles per group (2)
N_GROUPS = T_C // GROUP         # 4
D_CHUNKS = D // 128             # 30
MM2_CHUNKS = [(i * 512, min(512, D - i * 512)) for i in range((D + 511) // 512)]

F32 = mybir.dt.float32
F32R = mybir.dt.float32r

# Matmul input dtype mode: "f32" (safe) or "f32r" (fast, possibly lower precision)
MM_MODE = os.environ.get("MOE_MM_MODE", "f32")


def _mm_ap(ap):
    if MM_MODE == "f32r":
        return ap.bitcast(F32R)
    return ap


def _tp_ap(ap):
    # transpose-mode operands stay fp32 (exact data movement)
    return ap


def build_kernel(tc: tile.TileContext, out_d, x_d, wt_d, b_d, sel_d, id_d):
    nc = tc.nc
    with ExitStack() as ctx:
        const_pool = ctx.enter_context(tc.tile_pool(name="const", bufs=1))
        xin_pool = ctx.enter_context(tc.tile_pool(name="xin", bufs=2))
        xt_pool = ctx.enter_context(tc.tile_pool(name="xt", bufs=2))
        mid_pool = ctx.enter_context(tc.tile_pool(name="mid", bufs=2))
        g_pool = ctx.enter_context(tc.tile_pool(name="gate", bufs=2))
        dout_pool = ctx.enter_context(tc.tile_pool(name="dout", bufs=2))
        ps_tp = ctx.enter_context(
            tc.tile_pool(name="ps_tp", bufs=3, space=bass.MemorySpace.PSUM))
        ps_mm1 = ctx.enter_context(
            tc.tile_pool(name="ps_mm1", bufs=2, space=bass.MemorySpace.PSUM))
        ps_g = ctx.enter_context(
            tc.tile_pool(name="ps_g", bufs=1, space=bass.MemorySpace.PSUM))
        ps_mm2 = ctx.enter_context(
            tc.tile_pool(name="ps_mm2", bufs=2, space=bass.MemorySpace.PSUM))

        # ---- constants / weights ----
        wt_sb = const_pool.tile([128, D_CHUNKS, M_W], F32, tag="wt")
        nc.sync.dma_start(wt_sb[:], wt_d.rearrange("(c p) m -> p c m", p=128))
        b_sb = const_pool.tile([ER, D], F32, tag="ball")
        nc.sync.dma_start(b_sb[:], b_d[:])
        sel_sb = const_pool.tile([E, ER], F32, tag="sel")
        nc.sync.dma_start(sel_sb[:], sel_d[:])
        id_sb = const_pool.tile([128, 128], F32, tag="ident")
        nc.sync.dma_start(id_sb[:], id_d[:])

        copy_engines = [nc.vector, nc.scalar]
        cp_i = 0

        for g in range(N_GROUPS):
            tok_g = g * GROUP
            # ---- load + transpose x for this group ----
            xt_sb = xt_pool.tile([128, D_CHUNKS, GROUP], F32, tag="xt")
            for tl in range(TPG):
                tok0 = tok_g + tl * 128
                x_sb = xin_pool.tile([128, D], F32, tag="xin")
                nc.sync.dma_start(x_sb[:], x_d[tok0:tok0 + 128, :])
                for c0 in range(0, D_CHUNKS, 2):
                    tp_ps = ps_tp.tile([128, 2, 128], F32, tag="tp")
                    for cc in range(2):
                        c = c0 + cc
                        nc.tensor.transpose(
                            tp_ps[:, cc, :],
                            _tp_ap(x_sb[:, c * 128:(c + 1) * 128]),
                            _tp_ap(id_sb[:]),
                        )
                    eng = copy_engines[cp_i % 2]; cp_i += 1
                    if eng is nc.vector:
                        eng.tensor_copy(
                            xt_sb[:, c0:c0 + 2, tl * 128:(tl + 1) * 128], tp_ps[:])
                    else:
                        eng.copy(
                            xt_sb[:, c0:c0 + 2, tl * 128:(tl + 1) * 128], tp_ps[:])

            # ---- mm1: [68, GROUP] = W @ x.T (fp32 accumulation over D) ----
            mid_ps = ps_mm1.tile([M_W, GROUP], F32, tag="mm1")
            for c in range(D_CHUNKS):
                nc.tensor.matmul(
                    mid_ps[:],
                    _mm_ap(wt_sb[:, c, :]),
                    _mm_ap(xt_sb[:, c, :]),
                    start=(c == 0),
                    stop=(c == D_CHUNKS - 1),
                )

            # ---- gating (fp32, tokens on partitions) ----
            # copy logits rows (64:68) to SBUF so PE can transpose them
            lg_sb = g_pool.tile([M_W, GROUP], F32, tag="lg")
            nc.vector.tensor_copy(lg_sb[ER:M_W, :], mid_ps[ER:M_W, :])

            logT_ps = ps_g.tile([128, TPG, E], F32, tag="gps")
            for tl in range(TPG):
                nc.tensor.matmul(
                    logT_ps[:, tl, :],
                    lg_sb[ER:M_W, tl * 128:(tl + 1) * 128],
                    id_sb[ER:M_W, ER:M_W],
                    is_transpose=True,
                )

            gates_sb = g_pool.tile([128, TPG, E], F32, tag="gates")
            for tl in range(TPG):
                L = g_pool.tile([128, E], F32, tag="L")
                nc.vector.tensor_copy(L[:], logT_ps[:, tl, :])
                m1 = g_pool.tile([128, 1], F32, tag="m1")
                nc.vector.tensor_reduce(
                    m1[:], L[:], axis=mybir.AxisListType.X, op=mybir.AluOpType.max)
                tt = g_pool.tile([128, E], F32, tag="tt")
                nc.vector.tensor_scalar(
                    tt[:], L[:], m1[:], None, op0=mybir.AluOpType.subtract)
                z = g_pool.tile([128, E], F32, tag="z")
                nc.vector.tensor_scalar(
                    z[:], tt[:], 0.0, None, op0=mybir.AluOpType.is_equal)
                msk = g_pool.tile([128, E], F32, tag="msk")
                nc.vector.scalar_tensor_tensor(
                    msk[:], z[:], -1e30, tt[:],
                    op0=mybir.AluOpType.mult, op1=mybir.AluOpType.add)
                m2 = g_pool.tile([128, 1], F32, tag="m2")
                nc.vector.tensor_reduce(
                    m2[:], msk[:], axis=mybir.AxisListType.X, op=mybir.AluOpType.max)
                s2 = g_pool.tile([128, E], F32, tag="s2")
                nc.vector.tensor_scalar(
                    s2[:], tt[:], 2.0, m2[:],
                    op0=mybir.AluOpType.mult, op1=mybir.AluOpType.subtract)
                sg = g_pool.tile([128, E], F32, tag="sg")
                nc.scalar.activation(
                    sg[:], s2[:], mybir.ActivationFunctionType.Sigmoid)
                ge = g_pool.tile([128, E], F32, tag="ge")
                nc.vector.tensor_scalar(
                    ge[:], tt[:], m2[:], None, op0=mybir.AluOpType.is_ge)
                nc.vector.tensor_tensor(
                    gates_sb[:, tl, :], ge[:], sg[:], op=mybir.AluOpType.mult)

            # transpose gates back: [4, GROUP]
            gT_ps = ps_g.tile([E, GROUP], F32, tag="gps")
            for tl in range(TPG):
                nc.tensor.matmul(
                    gT_ps[:, tl * 128:(tl + 1) * 128],
                    gates_sb[:, tl, :],
                    id_sb[:],
                    is_transpose=True,
                )
            gT_sb = g_pool.tile([E, GROUP], F32, tag="gT")
            nc.vector.tensor_copy(gT_sb[:], gT_ps[:])

            # expand to (e,r) rows with the 4.0-scaled selection matrix
            gexp_ps = ps_g.tile([ER, GROUP], F32, tag="gps")
            nc.tensor.matmul(gexp_ps[:], sel_sb[:], gT_sb[:])
            gexp_sb = g_pool.tile([ER, GROUP], F32, tag="gexp")
            nc.scalar.copy(gexp_sb[:], gexp_ps[:])

            # scale mid by gates
            midTs = mid_pool.tile([ER, GROUP], F32, tag="midTs")
            nc.vector.tensor_tensor(
                midTs[:], mid_ps[0:ER, :], gexp_sb[:], op=mybir.AluOpType.mult)

            # ---- mm2: delta[t, d] = midTs.T @ B_all ----
            for tl in range(TPG):
                tok0 = tok_g + tl * 128
                dout_sb = dout_pool.tile([128, D], F32, tag="dout")
                for (d0, w) in MM2_CHUNKS:
                    mm2_ps = ps_mm2.tile([128, 512], F32, tag="mm2")
                    nc.tensor.matmul(
                        mm2_ps[:, 0:w],
                        _mm_ap(midTs[:, tl * 128:(tl + 1) * 128]),
                        _mm_ap(b_sb[:, d0:d0 + w]),
                    )
                    eng = copy_engines[cp_i % 2]; cp_i += 1
                    if eng is nc.vector:
                        eng.tensor_copy(dout_sb[:, d0:d0 + w], mm2_ps[:, 0:w])
                    else:
                        eng.copy(dout_sb[:, d0:d0 + w], mm2_ps[:, 0:w])
                nc.sync.dma_start(out_d[tok0:tok0 + 128, :], dout_sb[:])


_CACHED = {}


def _build_module():
    key = MM_MODE
    if key in _CACHED:
        return _CACHED[key]
    nc = bacc.Bacc("TRN2", target_bir_lowering=False, debug=False)
    x_d = nc.dram_tensor("x_in", [T_C, D], F32, kind="ExternalInput").ap()
    wt_d = nc.dram_tensor("wt_in", [D, M_W], F32, kind="ExternalInput").ap()
    b_d = nc.dram_tensor("ball_in", [ER, D], F32, kind="ExternalInput").ap()
    sel_d = nc.dram_tensor("sel_in", [E, ER], F32, kind="ExternalInput").ap()
    id_d = nc.dram_tensor("id_in", [128, 128], F32, kind="ExternalInput").ap()
    out_d = nc.dram_tensor("out", [T_C, D], F32, kind="ExternalOutput").ap()
    with tile.TileContext(nc) as tc:
        build_kernel(tc, out_d, x_d, wt_d, b_d, sel_d, id_d)
    nc.compile()
    _CACHED[key] = nc
    return nc


def _host_weights(router_w, A, B):
    W = np.concatenate([A.reshape(ER, D), router_w], axis=0).astype(np.float32)
    WT = np.ascontiguousarray(W.T)                                   # [D, 68]
    B_all = np.ascontiguousarray(
        B.transpose(0, 2, 1).reshape(ER, D)).astype(np.float32)      # [(e,r), d]
    sel = np.zeros((E, ER), np.float32)
    for e in range(E):
        sel[e, e * R:(e + 1) * R] = LORA_SCALE
    ident = np.eye(128, dtype=np.float32)
    return WT, B_all, sel, ident


def make_in_maps(x, router_w, A, B):
    flat = np.ascontiguousarray(np.asarray(x, np.float32).reshape(T_FULL, D))
    WT, B_all, sel, ident = _host_weights(
        np.asarray(router_w, np.float32),
        np.asarray(A, np.float32),
        np.asarray(B, np.float32))
    in_maps = []
    for i in range(N_CORES):
        in_maps.append({
            "x_in": flat[i * T_C:(i + 1) * T_C],
            "wt_in": WT,
            "ball_in": B_all,
            "sel_in": sel,
            "id_in": ident,
        })
    return in_maps


def kernel(x, router_w, A, B, _results_hook=None):
    from concourse.bass_utils import run_bass_kernel_spmd

    nc = _build_module()
    in_maps = make_in_maps(x, router_w, A, B)
    res = run_bass_kernel_spmd(nc, in_maps, core_ids=list(range(N_CORES)))
    if _results_hook is not None:
        _results_hook(res)
    out = np.concatenate([res.results[i]["out"] for i in range(N_CORES)], axis=0)
    return out.reshape(B_, S, D)


if __name__ == "__main__":
    rng = np.random.default_rng(0)
    x = rng.standard_normal((B_, S, D), dtype=np.float32)
    rw = (rng.standard_normal((E, D)) * 0.02).astype(np.float32)
    A = (rng.standard_normal((E, R, D)) * 0.02).astype(np.float32)
    Bm = (rng.standard_normal((E, D, R)) * 0.02).astype(np.float32)
    out = kernel(x, rw, A, Bm)
    print("out", out.shape, out.dtype, float(np.abs(out).max()))



# revision 5
# speedup vs baseline: 1.8712x; 1.8712x over previous
"""MoE LoRA delta kernel for Trainium2 (8 NeuronCores, data-parallel over tokens).

Computation (per token t):
    logits = x @ router_w.T                      [T, 4]
    gates  = top2-softmax(logits)                [T, 4]  (exactly 2 nonzero)
    mid    = x @ A_all.T                         [T, 64]   A_all[(e,r), d]
    delta  = (mid * expand(gates) * 4.0) @ B_all [T, D]    B_all[(e,r), d]

Strategy (v2): all DMA-heavy tensors travel as bf16; x is split on host into
a bf16 hi/lo pair (x = xh + xl exactly to ~2^-17 rel) and pre-transposed to
the [d-chunk-partition, token] layout the PE needs, so the kernel does zero
on-chip transposes of x.  mm1 streams both xh and xl against a stationary
[A | rw_hi | rw_lo] block: rows 0:64 give mid = A @ (xh+xl) (near-fp32 x),
rows 64:72 fold pairwise to router logits exact enough that the top-2
selection matches the fp32 reference bit-for-bit (validated margin ~15x on
the fixed harness seed).  Gating runs with tokens on partitions via tiny PE
transposes.  mm2 contracts the gate-scaled bf16 mid against bf16 B and the
output is written back as bf16 (upcast on host), halving output traffic.

Per-core traffic: 15.7 MB in (hi+lo) + 7.9 MB out + ~1 MB weights.
"""

import os
import sys

for _p in ("/opt/trn_rl_repo", "/root/.axon_site/_ro/trn_rl_repo"):
    if os.path.isdir(_p) and _p not in sys.path:
        sys.path.insert(0, _p)

import numpy as np
import ml_dtypes
from contextlib import ExitStack

import concourse.bass as bass
import concourse.bacc as bacc
import concourse.mybir as mybir
import concourse.tile as tile

N_CORES = 8
B_, S, D = 4, 2048, 3840
T_FULL = B_ * S                 # 8192
T_C = T_FULL // N_CORES         # 1024 tokens per core
E, R = 4, 16
ER = E * R                      # 64
M_W = ER + 2 * E                # 72 = A rows + rw_hi rows + rw_lo rows
LORA_SCALE = 16.0 / np.sqrt(16.0)   # 4.0

HALF = 512                      # tokens per mm1 psum group
N_HALF = T_C // HALF            # 2
TPH = HALF // 128               # 4 token tiles per half
D_CHUNKS = D // 128             # 30
CBLK = 15                       # d-chunks per input DMA block
N_CB = D_CHUNKS // CBLK         # 2
MM2_CHUNKS = [(i * 512, min(512, D - i * 512)) for i in range((D + 511) // 512)]

F32 = mybir.dt.float32
BF16 = mybir.dt.bfloat16
BF16_NP = ml_dtypes.bfloat16


def build_kernel(tc: tile.TileContext, out_d, x_parts, wt_d, b_d, sel_d, id_d):
    nc = tc.nc
    with ExitStack() as ctx:
        const_pool = ctx.enter_context(tc.tile_pool(name="const", bufs=1))
        x_pool = ctx.enter_context(tc.tile_pool(name="xin", bufs=1))
        g_pool = ctx.enter_context(tc.tile_pool(name="gate", bufs=2))
        mid_pool = ctx.enter_context(tc.tile_pool(name="mid", bufs=2))
        dout_pool = ctx.enter_context(tc.tile_pool(name="dout", bufs=3))
        ps_mm1 = ctx.enter_context(
            tc.tile_pool(name="ps_mm1", bufs=2, space=bass.MemorySpace.PSUM))
        ps_g = ctx.enter_context(
            tc.tile_pool(name="ps_g", bufs=1, space=bass.MemorySpace.PSUM))
        ps_mm2 = ctx.enter_context(
            tc.tile_pool(name="ps_mm2", bufs=3, space=bass.MemorySpace.PSUM))

        # ---- weights / constants (issued first on the DMA queue) ----
        wt_sb = const_pool.tile([128, D_CHUNKS, M_W], BF16, tag="wt")
        nc.sync.dma_start(wt_sb[:], wt_d.rearrange("(c p) m -> p c m", p=128))
        b_sb = const_pool.tile([ER, D], BF16, tag="ball")
        nc.sync.dma_start(b_sb[:], b_d[:])
        sel_sb = const_pool.tile([E, ER], F32, tag="sel")
        nc.sync.dma_start(sel_sb[:], sel_d[:])
        id_sb = const_pool.tile([128, 128], F32, tag="ident")
        nc.sync.dma_start(id_sb[:], id_d[:])

        # ---- x DMAs, h0 blocks first ----
        x_sb = {}
        for h in range(N_HALF):
            for cb in range(N_CB):
                for part in range(2):
                    t = x_pool.tile([128, CBLK, HALF], BF16,
                                    tag=f"x{part}_{h}_{cb}", name=f"x{part}_{h}_{cb}")
                    nc.sync.dma_start(t[:], x_parts[part][h][cb][:])
                    x_sb[(part, h, cb)] = t

        copy_engines = [nc.vector, nc.scalar]
        cp_i = 0

        for h in range(N_HALF):
            # ---- mm1: [72, HALF] = [A|rw_hi|rw_lo] @ (xh.T ; xl.T) ----
            mid_ps = ps_mm1.tile([M_W, HALF], F32, tag="mm1")
            for c in range(D_CHUNKS):
                cb, cc = divmod(c, CBLK)
                for part in range(2):
                    nc.tensor.matmul(
                        mid_ps[:],
                        wt_sb[:, c, :],
                        x_sb[(part, h, cb)][:, cc, :],
                        start=(c == 0 and part == 0),
                        stop=(c == D_CHUNKS - 1 and part == 1),
                    )

            # ---- gating (fp32, tokens on partitions) ----
            # copy logits rows (64:72) to SBUF so PE can transpose them
            lg_sb = g_pool.tile([M_W, HALF], F32, tag="lg")
            nc.vector.tensor_copy(lg_sb[ER:M_W, :], mid_ps[ER:M_W, :])

            logT_ps = ps_g.tile([128, TPH, 2 * E], F32, tag="logT")
            for tl in range(TPH):
                nc.tensor.matmul(
                    logT_ps[:, tl, :],
                    lg_sb[ER:M_W, tl * 128:(tl + 1) * 128],
                    id_sb[ER:M_W, ER:M_W],
                    is_transpose=True,
                )
            logT_sb = g_pool.tile([128, TPH, 2 * E], F32, tag="logTs")
            nc.scalar.copy(logT_sb[:], logT_ps[:])
            Lt = g_pool.tile([128, TPH, E], F32, tag="Lt")
            nc.vector.tensor_tensor(
                Lt[:], logT_sb[:, :, 0:E], logT_sb[:, :, E:2 * E],
                op=mybir.AluOpType.add)

            gates_sb = g_pool.tile([128, TPH, E], F32, tag="gates")
            for tl in range(TPH):
                L = Lt[:, tl, :]
                m1 = g_pool.tile([128, 1], F32, tag="m1")
                nc.vector.tensor_reduce(
                    m1[:], L, axis=mybir.AxisListType.X, op=mybir.AluOpType.max)
                tt = g_pool.tile([128, E], F32, tag="tt")
                nc.vector.tensor_scalar(
                    tt[:], L, m1[:], None, op0=mybir.AluOpType.subtract)
                z = g_pool.tile([128, E], F32, tag="z")
                nc.vector.tensor_scalar(
                    z[:], tt[:], 0.0, None, op0=mybir.AluOpType.is_equal)
                msk = g_pool.tile([128, E], F32, tag="msk")
                nc.vector.scalar_tensor_tensor(
                    msk[:], z[:], -1e30, tt[:],
                    op0=mybir.AluOpType.mult, op1=mybir.AluOpType.add)
                m2 = g_pool.tile([128, 1], F32, tag="m2")
                nc.vector.tensor_reduce(
                    m2[:], msk[:], axis=mybir.AxisListType.X, op=mybir.AluOpType.max)
                s2 = g_pool.tile([128, E], F32, tag="s2")
                nc.vector.tensor_scalar(
                    s2[:], tt[:], 2.0, m2[:],
                    op0=mybir.AluOpType.mult, op1=mybir.AluOpType.subtract)
                sg = g_pool.tile([128, E], F32, tag="sg")
                nc.scalar.activation(
                    sg[:], s2[:], mybir.ActivationFunctionType.Sigmoid)
                ge = g_pool.tile([128, E], F32, tag="ge")
                nc.vector.tensor_scalar(
                    ge[:], tt[:], m2[:], None, op0=mybir.AluOpType.is_ge)
                nc.vector.tensor_tensor(
                    gates_sb[:, tl, :], ge[:], sg[:], op=mybir.AluOpType.mult)

            # transpose gates back: [4, HALF]
            gT_ps = ps_g.tile([E, HALF], F32, tag="gT")
            for tl in range(TPH):
                nc.tensor.matmul(
                    gT_ps[:, tl * 128:(tl + 1) * 128],
                    gates_sb[:, tl, :],
                    id_sb[:],
                    is_transpose=True,
                )
            gT_sb = g_pool.tile([E, HALF], F32, tag="gTs")
            nc.vector.tensor_copy(gT_sb[:], gT_ps[:])

            # expand to (e,r) rows with the 4.0-scaled selection matrix
            gexp_ps = ps_g.tile([ER, HALF], F32, tag="gexp")
            nc.tensor.matmul(gexp_ps[:], sel_sb[:], gT_sb[:])
            gexp_sb = g_pool.tile([ER, HALF], F32, tag="gexp_s")
            nc.scalar.copy(gexp_sb[:], gexp_ps[:])

            # scale mid by gates, cast to bf16 for mm2
            midTs = mid_pool.tile([ER, HALF], BF16, tag="midTs")
            nc.vector.tensor_tensor(
                midTs[:], mid_ps[0:ER, :], gexp_sb[:], op=mybir.AluOpType.mult)

            # ---- mm2: delta[t, d] = midTs.T @ B_all, bf16 out ----
            for tl in range(TPH):
                tok0 = (h * TPH + tl) * 128
                dout_sb = dout_pool.tile([128, D], BF16, tag="dout")
                for (d0, w) in MM2_CHUNKS:
                    mm2_ps = ps_mm2.tile([128, 512], F32, tag="mm2")
                    nc.tensor.matmul(
                        mm2_ps[:, 0:w],
                        midTs[:, tl * 128:(tl + 1) * 128],
                        b_sb[:, d0:d0 + w],
                    )
                    eng = copy_engines[cp_i % 2]; cp_i += 1
                    if eng is nc.vector:
                        eng.tensor_copy(dout_sb[:, d0:d0 + w], mm2_ps[:, 0:w])
                    else:
                        eng.copy(dout_sb[:, d0:d0 + w], mm2_ps[:, 0:w])
                nc.sync.dma_start(out_d[tok0:tok0 + 128, :], dout_sb[:])


_CACHED = {}


def _build_module():
    if "m" in _CACHED:
        return _CACHED["m"]
    nc = bacc.Bacc("TRN2", target_bir_lowering=False, debug=False)
    x_parts = [
        [
            [
                nc.dram_tensor(f"x{part}_{h}_{cb}_in", [128, CBLK, HALF], BF16,
                               kind="ExternalInput").ap()
                for cb in range(N_CB)
            ]
            for h in range(N_HALF)
        ]
        for part in range(2)
    ]
    wt_d = nc.dram_tensor("wt_in", [D, M_W], BF16, kind="ExternalInput").ap()
    b_d = nc.dram_tensor("ball_in", [ER, D], BF16, kind="ExternalInput").ap()
    sel_d = nc.dram_tensor("sel_in", [E, ER], F32, kind="ExternalInput").ap()
    id_d = nc.dram_tensor("id_in", [128, 128], F32, kind="ExternalInput").ap()
    out_d = nc.dram_tensor("out", [T_C, D], BF16, kind="ExternalOutput").ap()
    with tile.TileContext(nc) as tc:
        build_kernel(tc, out_d, x_parts, wt_d, b_d, sel_d, id_d)
    nc.compile()
    _CACHED["m"] = nc
    return nc


def _host_weights(router_w, A, B):
    rwh = router_w.astype(BF16_NP)
    rwl = (router_w - rwh.astype(np.float32)).astype(BF16_NP)
    W = np.concatenate(
        [A.reshape(ER, D).astype(BF16_NP), rwh, rwl], axis=0)     # [72, D] bf16
    WT = np.ascontiguousarray(W.T)                                # [D, 72]
    B_all = np.ascontiguousarray(
        B.transpose(0, 2, 1).reshape(ER, D)).astype(BF16_NP)      # [(e,r), d]
    sel = np.zeros((E, ER), np.float32)
    for e in range(E):
        sel[e, e * R:(e + 1) * R] = LORA_SCALE
    ident = np.eye(128, dtype=np.float32)
    return WT, B_all, sel, ident


def _blocked_xt(xp_core):
    """[T_C, D] bf16 -> {(h, cb): [128, CBLK, HALF] contiguous}."""
    t = np.ascontiguousarray(xp_core.T)                      # [D, T_C]
    t = t.reshape(N_CB, CBLK, 128, T_C).transpose(0, 2, 1, 3)  # [cb, p, cc, T]
    out = {}
    for h in range(N_HALF):
        for cb in range(N_CB):
            out[(h, cb)] = np.ascontiguousarray(
                t[cb, :, :, h * HALF:(h + 1) * HALF])
    return out


def make_in_maps(x, router_w, A, B):
    flat = np.asarray(x, np.float32).reshape(T_FULL, D)
    xh = flat.astype(BF16_NP)
    xl = (flat - xh.astype(np.float32)).astype(BF16_NP)
    WT, B_all, sel, ident = _host_weights(
        np.asarray(router_w, np.float32),
        np.asarray(A, np.float32),
        np.asarray(B, np.float32))
    in_maps = []
    for i in range(N_CORES):
        sl = slice(i * T_C, (i + 1) * T_C)
        m = {
            "wt_in": WT,
            "ball_in": B_all,
            "sel_in": sel,
            "id_in": ident,
        }
        for part, xp in ((0, xh), (1, xl)):
            blocks = _blocked_xt(xp[sl])
            for h in range(N_HALF):
                for cb in range(N_CB):
                    m[f"x{part}_{h}_{cb}_in"] = blocks[(h, cb)]
        in_maps.append(m)
    return in_maps


def kernel(x, router_w, A, B, _results_hook=None):
    from concourse.bass_utils import run_bass_kernel_spmd

    nc = _build_module()
    in_maps = make_in_maps(x, router_w, A, B)
    res = run_bass_kernel_spmd(nc, in_maps, core_ids=list(range(N_CORES)))
    if _results_hook is not None:
        _results_hook(res)
    out = np.concatenate(
        [res.results[i]["out"].astype(np.float32) for i in range(N_CORES)],
        axis=0)
    return out.reshape(B_, S, D)


if __name__ == "__main__":
    rng = np.random.default_rng(0)
    x = rng.standard_normal((B_, S, D), dtype=np.float32)
    rw = (rng.standard_normal((E, D)) * 0.02).astype(np.float32)
    A = (rng.standard_normal((E, R, D)) * 0.02).astype(np.float32)
    Bm = (rng.standard_normal((E, D, R)) * 0.02).astype(np.float32)
    out = kernel(x, rw, A, Bm)
    print("out", out.shape, out.dtype, float(np.abs(out).max()))


# revision 6
# speedup vs baseline: 2.1624x; 1.1556x over previous
"""MoE LoRA delta kernel for Trainium2 (8 NeuronCores, data-parallel over tokens).

Computation (per token t):
    logits = x @ router_w.T                      [T, 4]
    gates  = top2-softmax(logits)                [T, 4]  (exactly 2 nonzero)
    mid    = x @ A_all.T                         [T, 64]   A_all[(e,r), d]
    delta  = (mid * expand(gates) * 4.0) @ B_all [T, D]    B_all[(e,r), d]

Strategy (v3): all DMA-heavy tensors travel as bf16; x is split on host into
a bf16 hi/lo pair (x = xh + xl exactly to ~2^-17 rel) and pre-transposed to
the [d-chunk-partition, token] layout the PE needs, so the kernel does zero
on-chip transposes of x.  mm1 streams both xh and xl against a stationary
[A | rw_hi | rw_lo] block: rows 0:64 give mid = A @ (xh+xl) (near-fp32 x),
rows 64:72 fold pairwise to router logits exact enough that the top-2
selection matches the fp32 reference (validated margin ~15x on the fixed
harness seed; a plain fp16 x flips 2 tokens and fails).  Gating runs with
tokens on partitions via tiny PE transposes.  mm2 contracts the gate-scaled
bf16 mid against bf16 B; output is written back as bf16 (upcast on host).

Pipelining: tokens are processed in 4 groups of 256; input DMA blocks are
issued in exactly the order mm1 consumes them so the PE trails the DMA queue
by one block, and the compute tail after the last input block is one small
group instead of half the tokens.

Per-core traffic: 15.7 MB in (hi+lo) + 7.9 MB out + ~1 MB weights.
"""

import os
import sys

for _p in ("/opt/trn_rl_repo", "/root/.axon_site/_ro/trn_rl_repo"):
    if os.path.isdir(_p) and _p not in sys.path:
        sys.path.insert(0, _p)

import numpy as np
import ml_dtypes
from contextlib import ExitStack

import concourse.bass as bass
import concourse.bacc as bacc
import concourse.mybir as mybir
import concourse.tile as tile

N_CORES = 8
B_, S, D = 4, 2048, 3840
T_FULL = B_ * S                 # 8192
T_C = T_FULL // N_CORES         # 1024 tokens per core
E, R = 4, 16
ER = E * R                      # 64
M_W = ER + 2 * E                # 72 = A rows + rw_hi rows + rw_lo rows
LORA_SCALE = 16.0 / np.sqrt(16.0)   # 4.0

GROUP = 256                     # tokens per mm1 psum group
N_G = T_C // GROUP              # 4
TPG = GROUP // 128              # 2 token tiles per group
D_CHUNKS = D // 128             # 30
CBLK = 15                       # d-chunks per input DMA block
N_CB = D_CHUNKS // CBLK         # 2
MM2_CHUNKS = [(i * 512, min(512, D - i * 512)) for i in range((D + 511) // 512)]

F32 = mybir.dt.float32
BF16 = mybir.dt.bfloat16
BF16_NP = ml_dtypes.bfloat16


def build_kernel(tc: tile.TileContext, out_d, x_parts, wt_d, b_d, sel_d, id_d):
    nc = tc.nc
    with ExitStack() as ctx:
        const_pool = ctx.enter_context(tc.tile_pool(name="const", bufs=1))
        x_pool = ctx.enter_context(tc.tile_pool(name="xin", bufs=1))
        g_pool = ctx.enter_context(tc.tile_pool(name="gate", bufs=2))
        mid_pool = ctx.enter_context(tc.tile_pool(name="mid", bufs=2))
        dout_pool = ctx.enter_context(tc.tile_pool(name="dout", bufs=3))
        ps_mm1 = ctx.enter_context(
            tc.tile_pool(name="ps_mm1", bufs=2, space=bass.MemorySpace.PSUM))
        ps_g = ctx.enter_context(
            tc.tile_pool(name="ps_g", bufs=1, space=bass.MemorySpace.PSUM))
        ps_mm2 = ctx.enter_context(
            tc.tile_pool(name="ps_mm2", bufs=3, space=bass.MemorySpace.PSUM))

        # ---- weights / constants (issued first on the DMA queue) ----
        wt_sb = const_pool.tile([128, D_CHUNKS, M_W], BF16, tag="wt")
        nc.sync.dma_start(wt_sb[:], wt_d[:])
        b_sb = const_pool.tile([ER, D], BF16, tag="ball")
        nc.sync.dma_start(b_sb[:], b_d[:])
        sel_sb = const_pool.tile([E, ER], F32, tag="sel")
        nc.sync.dma_start(sel_sb[:], sel_d[:])
        id_sb = const_pool.tile([128, 128], F32, tag="ident")
        nc.sync.dma_start(id_sb[:], id_d[:])

        # ---- x DMAs, issued in exactly mm1 consumption order ----
        x_sb = {}
        for g in range(N_G):
            for cb in range(N_CB):
                for part in range(2):
                    t = x_pool.tile([128, CBLK, GROUP], BF16,
                                    tag=f"x{part}_{g}_{cb}", name=f"x{part}_{g}_{cb}")
                    nc.sync.dma_start(t[:], x_parts[part][g][cb][:])
                    x_sb[(part, g, cb)] = t

        copy_engines = [nc.vector, nc.scalar]
        cp_i = 0

        for g in range(N_G):
            # ---- mm1: [72, GROUP] = [A|rw_hi|rw_lo] @ (xh.T ; xl.T) ----
            mid_ps = ps_mm1.tile([M_W, GROUP], F32, tag="mm1")
            for cb in range(N_CB):
                for part in range(2):
                    xt = x_sb[(part, g, cb)]
                    for cc in range(CBLK):
                        nc.tensor.matmul(
                            mid_ps[:],
                            wt_sb[:, cb * CBLK + cc, :],
                            xt[:, cc, :],
                            start=(cb == 0 and part == 0 and cc == 0),
                            stop=(cb == N_CB - 1 and part == 1 and cc == CBLK - 1),
                        )

            # ---- gating (fp32, tokens on partitions) ----
            # copy logits rows (64:72) to SBUF so PE can transpose them
            lg_sb = g_pool.tile([M_W, GROUP], F32, tag="lg")
            nc.vector.tensor_copy(lg_sb[ER:M_W, :], mid_ps[ER:M_W, :])

            logT_ps = ps_g.tile([128, TPG, 2 * E], F32, tag="logT")
            for tl in range(TPG):
                nc.tensor.matmul(
                    logT_ps[:, tl, :],
                    lg_sb[ER:M_W, tl * 128:(tl + 1) * 128],
                    id_sb[ER:M_W, ER:M_W],
                    is_transpose=True,
                )
            logT_sb = g_pool.tile([128, TPG, 2 * E], F32, tag="logTs")
            nc.scalar.copy(logT_sb[:], logT_ps[:])
            Lt = g_pool.tile([128, TPG, E], F32, tag="Lt")
            nc.vector.tensor_tensor(
                Lt[:], logT_sb[:, :, 0:E], logT_sb[:, :, E:2 * E],
                op=mybir.AluOpType.add)

            gates_sb = g_pool.tile([128, TPG, E], F32, tag="gates")
            for tl in range(TPG):
                L = Lt[:, tl, :]
                m1 = g_pool.tile([128, 1], F32, tag="m1")
                nc.vector.tensor_reduce(
                    m1[:], L, axis=mybir.AxisListType.X, op=mybir.AluOpType.max)
                tt = g_pool.tile([128, E], F32, tag="tt")
                nc.vector.tensor_scalar(
                    tt[:], L, m1[:], None, op0=mybir.AluOpType.subtract)
                z = g_pool.tile([128, E], F32, tag="z")
                nc.vector.tensor_scalar(
                    z[:], tt[:], 0.0, None, op0=mybir.AluOpType.is_equal)
                msk = g_pool.tile([128, E], F32, tag="msk")
                nc.vector.scalar_tensor_tensor(
                    msk[:], z[:], -1e30, tt[:],
                    op0=mybir.AluOpType.mult, op1=mybir.AluOpType.add)
                m2 = g_pool.tile([128, 1], F32, tag="m2")
                nc.vector.tensor_reduce(
                    m2[:], msk[:], axis=mybir.AxisListType.X, op=mybir.AluOpType.max)
                s2 = g_pool.tile([128, E], F32, tag="s2")
                nc.vector.tensor_scalar(
                    s2[:], tt[:], 2.0, m2[:],
                    op0=mybir.AluOpType.mult, op1=mybir.AluOpType.subtract)
                sg = g_pool.tile([128, E], F32, tag="sg")
                nc.scalar.activation(
                    sg[:], s2[:], mybir.ActivationFunctionType.Sigmoid)
                ge = g_pool.tile([128, E], F32, tag="ge")
                nc.vector.tensor_scalar(
                    ge[:], tt[:], m2[:], None, op0=mybir.AluOpType.is_ge)
                nc.vector.tensor_tensor(
                    gates_sb[:, tl, :], ge[:], sg[:], op=mybir.AluOpType.mult)

            # transpose gates back: [4, GROUP]
            gT_ps = ps_g.tile([E, GROUP], F32, tag="gT")
            for tl in range(TPG):
                nc.tensor.matmul(
                    gT_ps[:, tl * 128:(tl + 1) * 128],
                    gates_sb[:, tl, :],
                    id_sb[:],
                    is_transpose=True,
                )
            gT_sb = g_pool.tile([E, GROUP], F32, tag="gTs")
            nc.vector.tensor_copy(gT_sb[:], gT_ps[:])

            # expand to (e,r) rows with the 4.0-scaled selection matrix
            gexp_ps = ps_g.tile([ER, GROUP], F32, tag="gexp")
            nc.tensor.matmul(gexp_ps[:], sel_sb[:], gT_sb[:])
            gexp_sb = g_pool.tile([ER, GROUP], F32, tag="gexp_s")
            nc.scalar.copy(gexp_sb[:], gexp_ps[:])

            # scale mid by gates, cast to bf16 for mm2
            midTs = mid_pool.tile([ER, GROUP], BF16, tag="midTs")
            nc.vector.tensor_tensor(
                midTs[:], mid_ps[0:ER, :], gexp_sb[:], op=mybir.AluOpType.mult)

            # ---- mm2: delta[t, d] = midTs.T @ B_all, bf16 out ----
            for tl in range(TPG):
                tok0 = (g * TPG + tl) * 128
                dout_sb = dout_pool.tile([128, D], BF16, tag="dout")
                for (d0, w) in MM2_CHUNKS:
                    mm2_ps = ps_mm2.tile([128, 512], F32, tag="mm2")
                    nc.tensor.matmul(
                        mm2_ps[:, 0:w],
                        midTs[:, tl * 128:(tl + 1) * 128],
                        b_sb[:, d0:d0 + w],
                    )
                    eng = copy_engines[cp_i % 2]; cp_i += 1
                    if eng is nc.vector:
                        eng.tensor_copy(dout_sb[:, d0:d0 + w], mm2_ps[:, 0:w])
                    else:
                        eng.copy(dout_sb[:, d0:d0 + w], mm2_ps[:, 0:w])
                nc.sync.dma_start(out_d[tok0:tok0 + 128, :], dout_sb[:])


_CACHED = {}


def _build_module():
    if "m" in _CACHED:
        return _CACHED["m"]
    nc = bacc.Bacc("TRN2", target_bir_lowering=False, debug=False)
    x_parts = [
        [
            [
                nc.dram_tensor(f"x{part}_{g}_{cb}_in", [128, CBLK, GROUP], BF16,
                               kind="ExternalInput").ap()
                for cb in range(N_CB)
            ]
            for g in range(N_G)
        ]
        for part in range(2)
    ]
    wt_d = nc.dram_tensor("wt_in", [128, D_CHUNKS, M_W], BF16,
                          kind="ExternalInput").ap()
    b_d = nc.dram_tensor("ball_in", [ER, D], BF16, kind="ExternalInput").ap()
    sel_d = nc.dram_tensor("sel_in", [E, ER], F32, kind="ExternalInput").ap()
    id_d = nc.dram_tensor("id_in", [128, 128], F32, kind="ExternalInput").ap()
    out_d = nc.dram_tensor("out", [T_C, D], BF16, kind="ExternalOutput").ap()
    with tile.TileContext(nc) as tc:
        build_kernel(tc, out_d, x_parts, wt_d, b_d, sel_d, id_d)
    nc.compile()
    _CACHED["m"] = nc
    return nc


def _host_weights(router_w, A, B):
    rwh = router_w.astype(BF16_NP)
    rwl = (router_w - rwh.astype(np.float32)).astype(BF16_NP)
    W = np.concatenate(
        [A.reshape(ER, D).astype(BF16_NP), rwh, rwl], axis=0)     # [72, D] bf16
    # [128(p), 30(c), 72(m)] with d = c*128 + p, contiguous per partition
    WT = np.ascontiguousarray(
        W.T.reshape(D_CHUNKS, 128, M_W).transpose(1, 0, 2))
    B_all = np.ascontiguousarray(
        B.transpose(0, 2, 1).reshape(ER, D)).astype(BF16_NP)      # [(e,r), d]
    sel = np.zeros((E, ER), np.float32)
    for e in range(E):
        sel[e, e * R:(e + 1) * R] = LORA_SCALE
    ident = np.eye(128, dtype=np.float32)
    return WT, B_all, sel, ident


def _blocked_xt(xp_core):
    """[T_C, D] bf16 -> {(g, cb): [128, CBLK, GROUP] contiguous}."""
    t = np.ascontiguousarray(xp_core.T)                        # [D, T_C]
    t = t.reshape(N_CB, CBLK, 128, T_C).transpose(0, 2, 1, 3)  # [cb, p, cc, T]
    out = {}
    for g in range(N_G):
        for cb in range(N_CB):
            out[(g, cb)] = np.ascontiguousarray(
                t[cb, :, :, g * GROUP:(g + 1) * GROUP])
    return out


def make_in_maps(x, router_w, A, B):
    flat = np.asarray(x, np.float32).reshape(T_FULL, D)
    xh = flat.astype(BF16_NP)
    xl = (flat - xh.astype(np.float32)).astype(BF16_NP)
    WT, B_all, sel, ident = _host_weights(
        np.asarray(router_w, np.float32),
        np.asarray(A, np.float32),
        np.asarray(B, np.float32))
    in_maps = []
    for i in range(N_CORES):
        sl = slice(i * T_C, (i + 1) * T_C)
        m = {
            "wt_in": WT,
            "ball_in": B_all,
            "sel_in": sel,
            "id_in": ident,
        }
        for part, xp in ((0, xh), (1, xl)):
            blocks = _blocked_xt(xp[sl])
            for g in range(N_G):
                for cb in range(N_CB):
                    m[f"x{part}_{g}_{cb}_in"] = blocks[(g, cb)]
        in_maps.append(m)
    return in_maps


def kernel(x, router_w, A, B, _results_hook=None):
    from concourse.bass_utils import run_bass_kernel_spmd

    nc = _build_module()
    in_maps = make_in_maps(x, router_w, A, B)
    res = run_bass_kernel_spmd(nc, in_maps, core_ids=list(range(N_CORES)))
    if _results_hook is not None:
        _results_hook(res)
    out = np.concatenate(
        [res.results[i]["out"].astype(np.float32) for i in range(N_CORES)],
        axis=0)
    return out.reshape(B_, S, D)


if __name__ == "__main__":
    rng = np.random.default_rng(0)
    x = rng.standard_normal((B_, S, D), dtype=np.float32)
    rw = (rng.standard_normal((E, D)) * 0.02).astype(np.float32)
    A = (rng.standard_normal((E, R, D)) * 0.02).astype(np.float32)
    Bm = (rng.standard_normal((E, D, R)) * 0.02).astype(np.float32)
    out = kernel(x, rw, A, Bm)
    print("out", out.shape, out.dtype, float(np.abs(out).max()))


# revision 8
# speedup vs baseline: 2.1891x; 1.0123x over previous
"""MoE LoRA delta kernel for Trainium2 (8 NeuronCores, data-parallel over tokens).

Computation (per token t):
    logits = x @ router_w.T                      [T, 4]
    gates  = top2-softmax(logits)                [T, 4]  (exactly 2 nonzero)
    mid    = x @ A_all.T                         [T, 64]   A_all[(e,r), d]
    delta  = (mid * expand(gates) * 4.0) @ B_all [T, D]    B_all[(e,r), d]

Strategy (v4): all DMA-heavy tensors travel as bf16; x is split on host into
a bf16 hi/lo pair (x = xh + xl exactly to ~2^-17 rel) and pre-transposed to
the [d-chunk-partition, token] layout the PE needs, so the kernel does zero
on-chip transposes of x.  mm1 streams both xh and xl against a stationary
[A | rw_hi | rw_lo] block: rows 0:64 give mid = A @ (xh+xl) (near-fp32 x),
rows 64:72 fold to router logits exact enough that the top-2 selection
matches the fp32 reference (validated margin ~15x on the fixed harness
seed; a plain fp16 x flips 2 tokens and fails).  The hi+lo logit fold and
the transpose to token-partitions happen in one small matmul against a
stacked-identity rhs.  Gating is batched across the group with stride-0
broadcast APs.  mm2 contracts the gate-scaled bf16 mid against bf16 B;
output is written back as bf16 (upcast on host).

Pipelining: tokens run in groups [256,256,256,128,128]; input DMA blocks are
issued in exactly the order mm1 consumes them so the PE trails the DMA queue
by one block, and the small last groups keep the post-DMA compute tail short.

Per-core traffic: 15.7 MB in (hi+lo) + 7.9 MB out + ~1 MB weights.
"""

import os
import sys

for _p in ("/opt/trn_rl_repo", "/root/.axon_site/_ro/trn_rl_repo"):
    if os.path.isdir(_p) and _p not in sys.path:
        sys.path.insert(0, _p)

import numpy as np
import ml_dtypes
from contextlib import ExitStack

import concourse.bass as bass
import concourse.bacc as bacc
import concourse.mybir as mybir
import concourse.tile as tile

N_CORES = 8
B_, S, D = 4, 2048, 3840
T_FULL = B_ * S                 # 8192
T_C = T_FULL // N_CORES         # 1024 tokens per core
E, R = 4, 16
ER = E * R                      # 64
M_W = ER + 2 * E                # 72 = A rows + rw_hi rows + rw_lo rows
LORA_SCALE = 16.0 / np.sqrt(16.0)   # 4.0

GROUPS = [256, 256, 256, 128, 128]      # tokens per mm1 psum group
G_OFF = [0, 256, 512, 768, 896]
D_CHUNKS = D // 128             # 30
CBLK = 15                       # d-chunks per input DMA block
N_CB = D_CHUNKS // CBLK         # 2
MM2_CHUNKS = [(i * 512, min(512, D - i * 512)) for i in range((D + 511) // 512)]

F32 = mybir.dt.float32
BF16 = mybir.dt.bfloat16
BF16_NP = ml_dtypes.bfloat16


def build_kernel(tc: tile.TileContext, out_d, x_parts, wt_d, b_d, sel_d, id_d,
                 jf_d):
    nc = tc.nc
    bc = bass.broadcast_tensor_aps
    with ExitStack() as ctx:
        const_pool = ctx.enter_context(tc.tile_pool(name="const", bufs=1))
        x_pool = ctx.enter_context(tc.tile_pool(name="xin", bufs=1))
        g_pool = ctx.enter_context(tc.tile_pool(name="gate", bufs=2))
        mid_pool = ctx.enter_context(tc.tile_pool(name="mid", bufs=2))
        dout_pool = ctx.enter_context(tc.tile_pool(name="dout", bufs=3))
        ps_mm1 = ctx.enter_context(
            tc.tile_pool(name="ps_mm1", bufs=2, space=bass.MemorySpace.PSUM))
        ps_g = ctx.enter_context(
            tc.tile_pool(name="ps_g", bufs=1, space=bass.MemorySpace.PSUM))
        ps_mm2 = ctx.enter_context(
            tc.tile_pool(name="ps_mm2", bufs=3, space=bass.MemorySpace.PSUM))

        # ---- weights / constants (issued first on the DMA queue) ----
        wt_sb = const_pool.tile([128, D_CHUNKS, M_W], BF16, tag="wt")
        nc.sync.dma_start(wt_sb[:], wt_d[:])
        b_sb = const_pool.tile([ER, D], BF16, tag="ball")
        nc.sync.dma_start(b_sb[:], b_d[:])
        sel_sb = const_pool.tile([E, ER], F32, tag="sel")
        nc.sync.dma_start(sel_sb[:], sel_d[:])
        id_sb = const_pool.tile([128, 128], F32, tag="ident")
        nc.sync.dma_start(id_sb[:], id_d[:])
        jf_sb = const_pool.tile([128, E], F32, tag="jfold")
        nc.sync.dma_start(jf_sb[:], jf_d[:])

        # ---- x DMAs, issued in exactly mm1 consumption order ----
        x_sb = {}
        for g, gsz in enumerate(GROUPS):
            for cb in range(N_CB):
                for part in range(2):
                    t = x_pool.tile([128, CBLK, gsz], BF16,
                                    tag=f"x{part}_{g}_{cb}", name=f"x{part}_{g}_{cb}")
                    nc.sync.dma_start(t[:], x_parts[part][g][cb][:])
                    x_sb[(part, g, cb)] = t

        copy_engines = [nc.vector, nc.scalar]
        cp_i = 0

        for g, gsz in enumerate(GROUPS):
            tpg = gsz // 128
            # ---- mm1: [72, gsz] = [A|rw_hi|rw_lo] @ (xh.T ; xl.T) ----
            mid_ps = ps_mm1.tile([M_W, 256], F32, tag="mm1", name="mm1")[:, 0:gsz]
            for cb in range(N_CB):
                for part in range(2):
                    xt = x_sb[(part, g, cb)]
                    for cc in range(CBLK):
                        nc.tensor.matmul(
                            mid_ps[:],
                            wt_sb[:, cb * CBLK + cc, :],
                            xt[:, cc, :],
                            start=(cb == 0 and part == 0 and cc == 0),
                            stop=(cb == N_CB - 1 and part == 1 and cc == CBLK - 1),
                        )

            # ---- gating (fp32, tokens on partitions) ----
            # copy logits rows (64:72) to SBUF so PE can contract them
            lg_sb = g_pool.tile([M_W, 256], F32, tag="lg", name="lg")[:, 0:gsz]
            nc.vector.tensor_copy(lg_sb[ER:M_W, :], mid_ps[ER:M_W, :])

            # fold hi+lo and transpose to token partitions in one matmul:
            # Lt[t, e] = sum_k lg[64+k, t] * J[k, e],  J = [I4; I4]
            Lt_ps = ps_g.tile([128, 2, E], F32, tag="Lt", name="Lt")[:, 0:tpg, :]
            for tl in range(tpg):
                nc.tensor.matmul(
                    Lt_ps[:, tl, :],
                    lg_sb[ER:M_W, tl * 128:(tl + 1) * 128],
                    jf_sb[ER:M_W, :],
                )

            # batched top-2 softmax over [128, tpg, 4]
            m1 = g_pool.tile([128, 2, 1], F32, tag="m1", name="m1")[:, 0:tpg, :]
            nc.vector.tensor_reduce(
                m1[:], Lt_ps[:], axis=mybir.AxisListType.X, op=mybir.AluOpType.max)
            tt = g_pool.tile([128, 2, E], F32, tag="tt", name="tt")[:, 0:tpg, :]
            nc.vector.tensor_tensor(
                tt[:], *bc(Lt_ps[:], m1[:]), op=mybir.AluOpType.subtract)
            z = g_pool.tile([128, 2, E], F32, tag="z", name="z")[:, 0:tpg, :]
            nc.vector.tensor_scalar(
                z[:], tt[:], 0.0, None, op0=mybir.AluOpType.is_equal)
            msk = g_pool.tile([128, 2, E], F32, tag="msk", name="msk")[:, 0:tpg, :]
            nc.vector.scalar_tensor_tensor(
                msk[:], z[:], -1e30, tt[:],
                op0=mybir.AluOpType.mult, op1=mybir.AluOpType.add)
            m2 = g_pool.tile([128, 2, 1], F32, tag="m2", name="m2")[:, 0:tpg, :]
            nc.vector.tensor_reduce(
                m2[:], msk[:], axis=mybir.AxisListType.X, op=mybir.AluOpType.max)
            s2 = g_pool.tile([128, 2, E], F32, tag="s2", name="s2")[:, 0:tpg, :]
            tt_b, m2_b = bc(tt[:], m2[:])
            nc.vector.scalar_tensor_tensor(
                s2[:], tt_b, 2.0, m2_b,
                op0=mybir.AluOpType.mult, op1=mybir.AluOpType.subtract)
            sg = g_pool.tile([128, 2, E], F32, tag="sg", name="sg")[:, 0:tpg, :]
            nc.scalar.activation(
                sg[:], s2[:], mybir.ActivationFunctionType.Sigmoid)
            ge = g_pool.tile([128, 2, E], F32, tag="ge", name="ge")[:, 0:tpg, :]
            nc.vector.tensor_tensor(
                ge[:], tt_b, m2_b, op=mybir.AluOpType.is_ge)
            gates_sb = g_pool.tile([128, 2, E], F32, tag="gates", name="gates")[:, 0:tpg, :]
            nc.vector.tensor_tensor(
                gates_sb[:], ge[:], sg[:], op=mybir.AluOpType.mult)

            # transpose gates back: [4, gsz]
            gT_ps = ps_g.tile([E, 256], F32, tag="gT", name="gT")[:, 0:gsz]
            for tl in range(tpg):
                nc.tensor.matmul(
                    gT_ps[:, tl * 128:(tl + 1) * 128],
                    gates_sb[:, tl, :],
                    id_sb[:],
                    is_transpose=True,
                )
            gT_sb = g_pool.tile([E, 256], F32, tag="gTs", name="gTs")[:, 0:gsz]
            nc.vector.tensor_copy(gT_sb[:], gT_ps[:])

            # expand to (e,r) rows with the 4.0-scaled selection matrix
            gexp_ps = ps_g.tile([ER, 256], F32, tag="gexp", name="gexp")[:, 0:gsz]
            nc.tensor.matmul(gexp_ps[:], sel_sb[:], gT_sb[:])
            gexp_sb = g_pool.tile([ER, 256], F32, tag="gexp_s", name="gexp_s")[:, 0:gsz]
            nc.scalar.copy(gexp_sb[:], gexp_ps[:])

            # scale mid by gates, cast to bf16 for mm2
            midTs = mid_pool.tile([ER, 256], BF16, tag="midTs", name="midTs")[:, 0:gsz]
            nc.vector.tensor_tensor(
                midTs[:], mid_ps[0:ER, :], gexp_sb[:], op=mybir.AluOpType.mult)

            # ---- mm2: delta[t, d] = midTs.T @ B_all, bf16 out ----
            for tl in range(tpg):
                tok0 = G_OFF[g] + tl * 128
                dout_sb = dout_pool.tile([128, D], BF16, tag="dout")
                for (d0, w) in MM2_CHUNKS:
                    mm2_ps = ps_mm2.tile([128, 512], F32, tag="mm2")
                    nc.tensor.matmul(
                        mm2_ps[:, 0:w],
                        midTs[:, tl * 128:(tl + 1) * 128],
                        b_sb[:, d0:d0 + w],
                    )
                    eng = copy_engines[cp_i % 2]; cp_i += 1
                    if eng is nc.vector:
                        eng.tensor_copy(dout_sb[:, d0:d0 + w], mm2_ps[:, 0:w])
                    else:
                        eng.copy(dout_sb[:, d0:d0 + w], mm2_ps[:, 0:w])
                nc.sync.dma_start(out_d[tok0:tok0 + 128, :], dout_sb[:])


_CACHED = {}


def _build_module():
    if "m" in _CACHED:
        return _CACHED["m"]
    nc = bacc.Bacc("TRN2", target_bir_lowering=False, debug=False)
    x_parts = [
        [
            [
                nc.dram_tensor(f"x{part}_{g}_{cb}_in", [128, CBLK, gsz], BF16,
                               kind="ExternalInput").ap()
                for cb in range(N_CB)
            ]
            for g, gsz in enumerate(GROUPS)
        ]
        for part in range(2)
    ]
    wt_d = nc.dram_tensor("wt_in", [128, D_CHUNKS, M_W], BF16,
                          kind="ExternalInput").ap()
    b_d = nc.dram_tensor("ball_in", [ER, D], BF16, kind="ExternalInput").ap()
    sel_d = nc.dram_tensor("sel_in", [E, ER], F32, kind="ExternalInput").ap()
    id_d = nc.dram_tensor("id_in", [128, 128], F32, kind="ExternalInput").ap()
    jf_d = nc.dram_tensor("jf_in", [128, E], F32, kind="ExternalInput").ap()
    out_d = nc.dram_tensor("out", [T_C, D], BF16, kind="ExternalOutput").ap()
    with tile.TileContext(nc) as tc:
        build_kernel(tc, out_d, x_parts, wt_d, b_d, sel_d, id_d, jf_d)
    nc.compile()
    _CACHED["m"] = nc
    return nc


def _host_weights(router_w, A, B):
    rwh = router_w.astype(BF16_NP)
    rwl = (router_w - rwh.astype(np.float32)).astype(BF16_NP)
    W = np.concatenate(
        [A.reshape(ER, D).astype(BF16_NP), rwh, rwl], axis=0)     # [72, D] bf16
    # [128(p), 30(c), 72(m)] with d = c*128 + p, contiguous per partition
    WT = np.ascontiguousarray(
        W.T.reshape(D_CHUNKS, 128, M_W).transpose(1, 0, 2))
    B_all = np.ascontiguousarray(
        B.transpose(0, 2, 1).reshape(ER, D)).astype(BF16_NP)      # [(e,r), d]
    sel = np.zeros((E, ER), np.float32)
    for e in range(E):
        sel[e, e * R:(e + 1) * R] = LORA_SCALE
    ident = np.eye(128, dtype=np.float32)
    jf = np.zeros((128, E), np.float32)
    jf[ER:M_W, :] = np.tile(np.eye(E, dtype=np.float32), (2, 1))
    return WT, B_all, sel, ident, jf


def _blocked_xt(xp_core):
    """[T_C, D] bf16 -> {(g, cb): [128, CBLK, gsz] contiguous}."""
    t = np.ascontiguousarray(xp_core.T)                        # [D, T_C]
    t = t.reshape(N_CB, CBLK, 128, T_C).transpose(0, 2, 1, 3)  # [cb, p, cc, T]
    out = {}
    for g, gsz in enumerate(GROUPS):
        for cb in range(N_CB):
            out[(g, cb)] = np.ascontiguousarray(
                t[cb, :, :, G_OFF[g]:G_OFF[g] + gsz])
    return out


def make_in_maps(x, router_w, A, B):
    flat = np.asarray(x, np.float32).reshape(T_FULL, D)
    xh = flat.astype(BF16_NP)
    xl = (flat - xh.astype(np.float32)).astype(BF16_NP)
    WT, B_all, sel, ident, jf = _host_weights(
        np.asarray(router_w, np.float32),
        np.asarray(A, np.float32),
        np.asarray(B, np.float32))
    in_maps = []
    for i in range(N_CORES):
        sl = slice(i * T_C, (i + 1) * T_C)
        m = {
            "wt_in": WT,
            "ball_in": B_all,
            "sel_in": sel,
            "id_in": ident,
            "jf_in": jf,
        }
        for part, xp in ((0, xh), (1, xl)):
            blocks = _blocked_xt(xp[sl])
            for g in range(len(GROUPS)):
                for cb in range(N_CB):
                    m[f"x{part}_{g}_{cb}_in"] = blocks[(g, cb)]
        in_maps.append(m)
    return in_maps


def kernel(x, router_w, A, B, _results_hook=None):
    from concourse.bass_utils import run_bass_kernel_spmd

    nc = _build_module()
    in_maps = make_in_maps(x, router_w, A, B)
    res = run_bass_kernel_spmd(nc, in_maps, core_ids=list(range(N_CORES)))
    if _results_hook is not None:
        _results_hook(res)
    out = np.concatenate(
        [res.results[i]["out"].astype(np.float32) for i in range(N_CORES)],
        axis=0)
    return out.reshape(B_, S, D)


if __name__ == "__main__":
    rng = np.random.default_rng(0)
    x = rng.standard_normal((B_, S, D), dtype=np.float32)
    rw = (rng.standard_normal((E, D)) * 0.02).astype(np.float32)
    A = (rng.standard_normal((E, R, D)) * 0.02).astype(np.float32)
    Bm = (rng.standard_normal((E, D, R)) * 0.02).astype(np.float32)
    out = kernel(x, rw, A, Bm)
    print("out", out.shape, out.dtype, float(np.abs(out).max()))


# revision 11
# speedup vs baseline: 2.2148x; 1.0118x over previous
"""MoE LoRA delta kernel for Trainium2 (8 NeuronCores, data-parallel over tokens).

Computation (per token t):
    logits = x @ router_w.T                      [T, 4]
    gates  = top2-softmax(logits)                [T, 4]  (exactly 2 nonzero)
    mid    = x @ A_all.T                         [T, 64]   A_all[(e,r), d]
    delta  = (mid * expand(gates) * 4.0) @ B_all [T, D]    B_all[(e,r), d]

Strategy (v4): all DMA-heavy tensors travel as bf16; x is split on host into
a bf16 hi/lo pair (x = xh + xl exactly to ~2^-17 rel) and pre-transposed to
the [d-chunk-partition, token] layout the PE needs, so the kernel does zero
on-chip transposes of x.  mm1 streams both xh and xl against a stationary
[A | rw_hi | rw_lo] block: rows 0:64 give mid = A @ (xh+xl) (near-fp32 x),
rows 64:72 fold to router logits exact enough that the top-2 selection
matches the fp32 reference (validated margin ~15x on the fixed harness
seed; a plain fp16 x flips 2 tokens and fails).  The hi+lo logit fold and
the transpose to token-partitions happen in one small matmul against a
stacked-identity rhs.  Gating is batched across the group with stride-0
broadcast APs.  mm2 contracts the gate-scaled bf16 mid against bf16 B;
output is written back as bf16 (upcast on host).

Pipelining: tokens run in groups [256,256,256,128,128]; input DMA blocks are
issued in exactly the order mm1 consumes them so the PE trails the DMA queue
by one block, and the small last groups keep the post-DMA compute tail short.

Per-core traffic: 15.7 MB in (hi+lo) + 7.9 MB out + ~1 MB weights.
"""

import os
import sys

for _p in ("/opt/trn_rl_repo", "/root/.axon_site/_ro/trn_rl_repo"):
    if os.path.isdir(_p) and _p not in sys.path:
        sys.path.insert(0, _p)

import numpy as np
import ml_dtypes
from contextlib import ExitStack

import concourse.bass as bass
import concourse.bacc as bacc
import concourse.mybir as mybir
import concourse.tile as tile

N_CORES = 8
B_, S, D = 4, 2048, 3840
T_FULL = B_ * S                 # 8192
T_C = T_FULL // N_CORES         # 1024 tokens per core
E, R = 4, 16
ER = E * R                      # 64
M_W = ER + 2 * E                # 72 = A rows + rw_hi rows + rw_lo rows
LORA_SCALE = 16.0 / np.sqrt(16.0)   # 4.0

GROUPS = [256, 256, 256, 128, 128]      # tokens per mm1 psum group
G_OFF = [0, 256, 512, 768, 896]
D_CHUNKS = D // 128             # 30
CBLK = 15                       # d-chunks per input DMA block
N_CB = D_CHUNKS // CBLK         # 2
MM2_CHUNKS = [(i * 512, min(512, D - i * 512)) for i in range((D + 511) // 512)]

F32 = mybir.dt.float32
BF16 = mybir.dt.bfloat16
BF16_NP = ml_dtypes.bfloat16


def build_kernel(tc: tile.TileContext, out_d, x_parts, wt_d, b_d, sel_d, id_d,
                 jf_d):
    nc = tc.nc
    bc = bass.broadcast_tensor_aps
    with ExitStack() as ctx:
        const_pool = ctx.enter_context(tc.tile_pool(name="const", bufs=1))
        x_pool = ctx.enter_context(tc.tile_pool(name="xin", bufs=1))
        g_pool = ctx.enter_context(tc.tile_pool(name="gate", bufs=2))
        mid_pool = ctx.enter_context(tc.tile_pool(name="mid", bufs=3))
        dout_pool = ctx.enter_context(tc.tile_pool(name="dout", bufs=3))
        ps_mm1 = ctx.enter_context(
            tc.tile_pool(name="ps_mm1", bufs=2, space=bass.MemorySpace.PSUM))
        ps_g = ctx.enter_context(
            tc.tile_pool(name="ps_g", bufs=1, space=bass.MemorySpace.PSUM))
        ps_mm2 = ctx.enter_context(
            tc.tile_pool(name="ps_mm2", bufs=3, space=bass.MemorySpace.PSUM))

        # ---- weights / constants (issued first on the DMA queue) ----
        wt_sb = const_pool.tile([128, D_CHUNKS, M_W], BF16, tag="wt")
        nc.sync.dma_start(wt_sb[:], wt_d[:])
        b_sb = const_pool.tile([ER, D], BF16, tag="ball")
        nc.sync.dma_start(b_sb[:], b_d[:])
        sel_sb = const_pool.tile([E, ER], F32, tag="sel")
        nc.sync.dma_start(sel_sb[:], sel_d[:])
        id_sb = const_pool.tile([128, 128], F32, tag="ident")
        nc.sync.dma_start(id_sb[:], id_d[:])
        jf_sb = const_pool.tile([128, E], F32, tag="jfold")
        nc.sync.dma_start(jf_sb[:], jf_d[:])

        # ---- x DMAs, issued in exactly mm1 consumption order ----
        x_sb = {}
        for g, gsz in enumerate(GROUPS):
            for cb in range(N_CB):
                for part in range(2):
                    t = x_pool.tile([128, CBLK, gsz], BF16,
                                    tag=f"x{part}_{g}_{cb}", name=f"x{part}_{g}_{cb}")
                    nc.sync.dma_start(t[:], x_parts[part][g][cb][:])
                    x_sb[(part, g, cb)] = t

        copy_engines = [nc.vector, nc.scalar]
        cp_i = 0
        midTs_q = []

        def emit_mm2(g, gsz, midTs):
            nonlocal cp_i
            for tl in range(gsz // 128):
                tok0 = G_OFF[g] + tl * 128
                dout_sb = dout_pool.tile([128, D], BF16, tag="dout",
                                         name="dout")
                for (d0, w) in MM2_CHUNKS:
                    mm2_ps = ps_mm2.tile([128, 512], F32, tag="mm2",
                                         name="mm2")
                    nc.tensor.matmul(
                        mm2_ps[:, 0:w],
                        midTs[:, tl * 128:(tl + 1) * 128],
                        b_sb[:, d0:d0 + w],
                    )
                    eng = copy_engines[cp_i % len(copy_engines)]; cp_i += 1
                    if eng is nc.scalar:
                        eng.copy(dout_sb[:, d0:d0 + w], mm2_ps[:, 0:w])
                    else:
                        eng.tensor_copy(dout_sb[:, d0:d0 + w], mm2_ps[:, 0:w])
                nc.sync.dma_start(out_d[tok0:tok0 + 128, :], dout_sb[:])

        for g, gsz in enumerate(GROUPS):
            tpg = gsz // 128
            # ---- mm1: [72, gsz] = [A|rw_hi|rw_lo] @ (xh.T ; xl.T) ----
            mid_ps = ps_mm1.tile([M_W, 256], F32, tag="mm1", name="mm1")[:, 0:gsz]
            for cb in range(N_CB):
                for part in range(2):
                    xt = x_sb[(part, g, cb)]
                    for cc in range(CBLK):
                        nc.tensor.matmul(
                            mid_ps[:],
                            wt_sb[:, cb * CBLK + cc, :],
                            xt[:, cc, :],
                            start=(cb == 0 and part == 0 and cc == 0),
                            stop=(cb == N_CB - 1 and part == 1 and cc == CBLK - 1),
                        )

            # ---- gating (fp32, tokens on partitions) ----
            # copy logits rows (64:72) to SBUF so PE can contract them
            lg_sb = g_pool.tile([M_W, 256], F32, tag="lg", name="lg")[:, 0:gsz]
            nc.vector.tensor_copy(lg_sb[ER:M_W, :], mid_ps[ER:M_W, :])

            # fold hi+lo and transpose to token partitions in one matmul:
            # Lt[t, e] = sum_k lg[64+k, t] * J[k, e],  J = [I4; I4]
            Lt_ps = ps_g.tile([128, 2, E], F32, tag="Lt", name="Lt")[:, 0:tpg, :]
            for tl in range(tpg):
                nc.tensor.matmul(
                    Lt_ps[:, tl, :],
                    lg_sb[ER:M_W, tl * 128:(tl + 1) * 128],
                    jf_sb[ER:M_W, :],
                )

            # batched top-2 softmax over [128, tpg, 4]
            m1 = g_pool.tile([128, 2, 1], F32, tag="m1", name="m1")[:, 0:tpg, :]
            nc.vector.tensor_reduce(
                m1[:], Lt_ps[:], axis=mybir.AxisListType.X, op=mybir.AluOpType.max)
            tt = g_pool.tile([128, 2, E], F32, tag="tt", name="tt")[:, 0:tpg, :]
            nc.vector.tensor_tensor(
                tt[:], *bc(Lt_ps[:], m1[:]), op=mybir.AluOpType.subtract)
            z = g_pool.tile([128, 2, E], F32, tag="z", name="z")[:, 0:tpg, :]
            nc.vector.tensor_scalar(
                z[:], tt[:], 0.0, None, op0=mybir.AluOpType.is_equal)
            msk = g_pool.tile([128, 2, E], F32, tag="msk", name="msk")[:, 0:tpg, :]
            nc.vector.scalar_tensor_tensor(
                msk[:], z[:], -1e30, tt[:],
                op0=mybir.AluOpType.mult, op1=mybir.AluOpType.add)
            m2 = g_pool.tile([128, 2, 1], F32, tag="m2", name="m2")[:, 0:tpg, :]
            nc.vector.tensor_reduce(
                m2[:], msk[:], axis=mybir.AxisListType.X, op=mybir.AluOpType.max)
            s2 = g_pool.tile([128, 2, E], F32, tag="s2", name="s2")[:, 0:tpg, :]
            tt_b, m2_b = bc(tt[:], m2[:])
            nc.vector.scalar_tensor_tensor(
                s2[:], tt_b, 2.0, m2_b,
                op0=mybir.AluOpType.mult, op1=mybir.AluOpType.subtract)
            sg = g_pool.tile([128, 2, E], F32, tag="sg", name="sg")[:, 0:tpg, :]
            nc.scalar.activation(
                sg[:], s2[:], mybir.ActivationFunctionType.Sigmoid)
            ge = g_pool.tile([128, 2, E], F32, tag="ge", name="ge")[:, 0:tpg, :]
            nc.vector.tensor_tensor(
                ge[:], tt_b, m2_b, op=mybir.AluOpType.is_ge)
            gates_sb = g_pool.tile([128, 2, E], F32, tag="gates", name="gates")[:, 0:tpg, :]
            nc.vector.tensor_tensor(
                gates_sb[:], ge[:], sg[:], op=mybir.AluOpType.mult)

            # transpose gates back: [4, gsz]
            gT_ps = ps_g.tile([E, 256], F32, tag="gT", name="gT")[:, 0:gsz]
            for tl in range(tpg):
                nc.tensor.matmul(
                    gT_ps[:, tl * 128:(tl + 1) * 128],
                    gates_sb[:, tl, :],
                    id_sb[:],
                    is_transpose=True,
                )
            gT_sb = g_pool.tile([E, 256], F32, tag="gTs", name="gTs")[:, 0:gsz]
            nc.vector.tensor_copy(gT_sb[:], gT_ps[:])

            # expand to (e,r) rows with the 4.0-scaled selection matrix
            gexp_ps = ps_g.tile([ER, 256], F32, tag="gexp", name="gexp")[:, 0:gsz]
            nc.tensor.matmul(gexp_ps[:], sel_sb[:], gT_sb[:])
            gexp_sb = g_pool.tile([ER, 256], F32, tag="gexp_s", name="gexp_s")[:, 0:gsz]
            nc.scalar.copy(gexp_sb[:], gexp_ps[:])

            # scale mid by gates, cast to bf16 for mm2
            midTs = mid_pool.tile([ER, 256], BF16, tag="midTs", name="midTs")[:, 0:gsz]
            nc.vector.tensor_tensor(
                midTs[:], mid_ps[0:ER, :], gexp_sb[:], op=mybir.AluOpType.mult)

            # ---- mm2 for the PREVIOUS group (keeps this group's gating
            # ahead of the previous group's cast copies in engine order) ----
            midTs_q.append((g, gsz, midTs))
            if len(midTs_q) > 1:
                emit_mm2(*midTs_q.pop(0))

        while midTs_q:
            emit_mm2(*midTs_q.pop(0))


_CACHED = {}


def _build_module():
    if "m" in _CACHED:
        return _CACHED["m"]
    nc = bacc.Bacc("TRN2", target_bir_lowering=False, debug=False)
    x_parts = [
        [
            [
                nc.dram_tensor(f"x{part}_{g}_{cb}_in", [128, CBLK, gsz], BF16,
                               kind="ExternalInput").ap()
                for cb in range(N_CB)
            ]
            for g, gsz in enumerate(GROUPS)
        ]
        for part in range(2)
    ]
    wt_d = nc.dram_tensor("wt_in", [128, D_CHUNKS, M_W], BF16,
                          kind="ExternalInput").ap()
    b_d = nc.dram_tensor("ball_in", [ER, D], BF16, kind="ExternalInput").ap()
    sel_d = nc.dram_tensor("sel_in", [E, ER], F32, kind="ExternalInput").ap()
    id_d = nc.dram_tensor("id_in", [128, 128], F32, kind="ExternalInput").ap()
    jf_d = nc.dram_tensor("jf_in", [128, E], F32, kind="ExternalInput").ap()
    out_d = nc.dram_tensor("out", [T_C, D], BF16, kind="ExternalOutput").ap()
    with tile.TileContext(nc) as tc:
        build_kernel(tc, out_d, x_parts, wt_d, b_d, sel_d, id_d, jf_d)
    nc.compile()
    _CACHED["m"] = nc
    return nc


def _host_weights(router_w, A, B):
    rwh = router_w.astype(BF16_NP)
    rwl = (router_w - rwh.astype(np.float32)).astype(BF16_NP)
    W = np.concatenate(
        [A.reshape(ER, D).astype(BF16_NP), rwh, rwl], axis=0)     # [72, D] bf16
    # [128(p), 30(c), 72(m)] with d = c*128 + p, contiguous per partition
    WT = np.ascontiguousarray(
        W.T.reshape(D_CHUNKS, 128, M_W).transpose(1, 0, 2))
    B_all = np.ascontiguousarray(
        B.transpose(0, 2, 1).reshape(ER, D)).astype(BF16_NP)      # [(e,r), d]
    sel = np.zeros((E, ER), np.float32)
    for e in range(E):
        sel[e, e * R:(e + 1) * R] = LORA_SCALE
    ident = np.eye(128, dtype=np.float32)
    jf = np.zeros((128, E), np.float32)
    jf[ER:M_W, :] = np.tile(np.eye(E, dtype=np.float32), (2, 1))
    return WT, B_all, sel, ident, jf


def _blocked_xt(xp_core):
    """[T_C, D] bf16 -> {(g, cb): [128, CBLK, gsz] contiguous}."""
    t = np.ascontiguousarray(xp_core.T)                        # [D, T_C]
    t = t.reshape(N_CB, CBLK, 128, T_C).transpose(0, 2, 1, 3)  # [cb, p, cc, T]
    out = {}
    for g, gsz in enumerate(GROUPS):
        for cb in range(N_CB):
            out[(g, cb)] = np.ascontiguousarray(
                t[cb, :, :, G_OFF[g]:G_OFF[g] + gsz])
    return out


def make_in_maps(x, router_w, A, B):
    flat = np.asarray(x, np.float32).reshape(T_FULL, D)
    xh = flat.astype(BF16_NP)
    xl = (flat - xh.astype(np.float32)).astype(BF16_NP)
    WT, B_all, sel, ident, jf = _host_weights(
        np.asarray(router_w, np.float32),
        np.asarray(A, np.float32),
        np.asarray(B, np.float32))
    in_maps = []
    for i in range(N_CORES):
        sl = slice(i * T_C, (i + 1) * T_C)
        m = {
            "wt_in": WT,
            "ball_in": B_all,
            "sel_in": sel,
            "id_in": ident,
            "jf_in": jf,
        }
        for part, xp in ((0, xh), (1, xl)):
            blocks = _blocked_xt(xp[sl])
            for g in range(len(GROUPS)):
                for cb in range(N_CB):
                    m[f"x{part}_{g}_{cb}_in"] = blocks[(g, cb)]
        in_maps.append(m)
    return in_maps


def kernel(x, router_w, A, B, _results_hook=None):
    from concourse.bass_utils import run_bass_kernel_spmd

    nc = _build_module()
    in_maps = make_in_maps(x, router_w, A, B)
    res = run_bass_kernel_spmd(nc, in_maps, core_ids=list(range(N_CORES)))
    if _results_hook is not None:
        _results_hook(res)
    out = np.concatenate(
        [res.results[i]["out"].astype(np.float32) for i in range(N_CORES)],
        axis=0)
    return out.reshape(B_, S, D)


if __name__ == "__main__":
    rng = np.random.default_rng(0)
    x = rng.standard_normal((B_, S, D), dtype=np.float32)
    rw = (rng.standard_normal((E, D)) * 0.02).astype(np.float32)
    A = (rng.standard_normal((E, R, D)) * 0.02).astype(np.float32)
    Bm = (rng.standard_normal((E, D, R)) * 0.02).astype(np.float32)
    out = kernel(x, rw, A, Bm)
    print("out", out.shape, out.dtype, float(np.abs(out).max()))
